# revision 4
# baseline (speedup 1.0000x reference)
"""CLVP attention kernel for 8 Trainium2 NeuronCores.

Sharding: core c = 2*b + hg handles batch b (2048 tokens) and head-group hg
(8 of 16 heads).  Each core computes q/k/v projections for its heads, partial
rotary, attention, and a partial output projection over its heads' dims; the
host sums the two head-group partials per batch and adds the bias.

On-core layout highlights:
  - projections run in "natural" [token, dim] layout so partial rotary is a
    few strided DVE ops; q/k are then PE-transposed to [dim, token] for QK^T.
  - scores are computed transposed (S^T = [k_tok, q_tok]) so softmax's
    denominator and attn@v both contract over the partition axis on PE.
  - exp via ScalarE from PSUM, no max-subtraction (logits are O(1) here).
  - the v matmul uses a ones-extended stationary operand [v | 1] so row 64 of
    the accumulator is the softmax denominator for free.
  - everything runs in float32r (full-speed PE fp32 mode, ~1e-4 rounding).
"""

import numpy as np

import concourse.bass as bass
import concourse.tile as tile
from concourse import bacc, mybir
from concourse.bass_utils import run_bass_kernel_spmd

B, S, E, H, D, ROT = 4, 2048, 1024, 16, 64, 32
HLOC = 8            # heads per core
HS = HLOC * D       # 512 head dims per core
N_CORES = 8
KE = E // 128       # 8 contraction tiles for projections
TT = S // 128       # 16 token tiles
QC = S // 512       # 4 q chunks
KC = S // 128       # 16 k chunks
PT = HS // 128      # 4 pair-tiles (2 heads each)

f32 = mybir.dt.float32
f32r = mybir.dt.float32r
FT = mybir.ActivationFunctionType


def _emit(nc, tc, ctx, t):
    hidT, wq, wk, wv, m1, cmat, smat, ident_in, ones_in, part = t
    w_dram = {"q": wq, "k": wk, "v": wv}

    const = ctx.enter_context(tc.tile_pool(name="const", bufs=1))
    ident = const.tile([128, 128], f32r)
    nc.sync.dma_start(ident[:], ident_in.ap())
    ones_t = const.tile([128, 64], f32r)
    nc.sync.dma_start(ones_t[:], ones_in.ap())
    c_sb = const.tile([128, TT * 64], f32)
    nc.sync.dma_start(
        c_sb[:].rearrange("p (t d) -> p t d", d=64),
        cmat.ap().rearrange("(t p) d -> p t d", p=128),
    )
    s_sb = const.tile([128, TT * 32], f32)
    nc.sync.dma_start(
        s_sb[:].rearrange("p (t d) -> p t d", d=32),
        smat.ap().rearrange("(t p) d -> p t d", p=128),
    )
    # v_ext: [k-tile kc][head h][65] ; col 64 of each slot stays 1.0
    vext = const.tile([128, KC * HLOC * 65], f32r)
    nc.vector.tensor_copy(
        vext[:].rearrange("p (s c) -> p s c", c=65)[:, :, 64:65],
        ones_t[:, 0:1].rearrange("p (o c) -> p o c", o=1).broadcast_to(
            [128, KC * HLOC, 1]
        ),
    )
    kT = const.tile([128, PT * S], f32r)   # [pair-tile][token]
    qT = const.tile([128, PT * S], f32r)

    # ---------------- phase 1: projections + rotary + transpose ----------
    with tc.tile_pool(name="wts", bufs=1) as w_pool, tc.tile_pool(
        name="hid", bufs=16
    ) as hid_pool, tc.tile_pool(name="xnat", bufs=4) as xnat_pool, tc.tile_pool(
        name="tmp", bufs=4
    ) as tmp_pool, tc.tile_pool(
        name="pproj", bufs=4, space="PSUM"
    ) as proj_psum, tc.tile_pool(
        name="ptr", bufs=2, space="PSUM"
    ) as tr_psum:
        w_sb = {}
        for name in ("q", "k", "v"):
            wt = w_pool.tile([128, KE * HS], f32r, name=f"w{name}", tag=f"w{name}")
            nc.sync.dma_start(
                wt[:].rearrange("p (k n) -> p k n", n=HS),
                w_dram[name].ap().rearrange("(k p) n -> p k n", p=128),
            )
            w_sb[name] = wt

        for c8 in range(S // 256):
            hid_sl = []
            for k in range(KE):
                ht = hid_pool.tile([128, 256], f32r, tag="hid")
                nc.sync.dma_start(
                    ht[:], hidT.ap()[128 * k : 128 * (k + 1), 256 * c8 : 256 * c8 + 256]
                )
                hid_sl.append(ht)
            for X in ("k", "v", "q"):
                for t2 in range(2):
                    tt = 2 * c8 + t2
                    ps = proj_psum.tile([128, HS], f32, tag="pp")
                    for k in range(KE):
                        nc.tensor.matmul(
                            ps[:],
                            hid_sl[k][:, 128 * t2 : 128 * t2 + 128],
                            w_sb[X][:, HS * k : HS * (k + 1)],
                            start=(k == 0),
                            stop=(k == KE - 1),
                        )
                    psv = ps[:].rearrange("p (h d) -> p h d", d=64)
                    if X == "v":
                        blk = vext[:, 520 * tt : 520 * (tt + 1)]
                        outv = blk.rearrange("p (h c) -> p h c", c=65)[:, :, 0:64]
                    else:
                        xn = xnat_pool.tile([128, HS], f32r, tag="xn")
                        outv = xn[:].rearrange("p (h d) -> p h d", d=64)
                    cb = (
                        c_sb[:, 64 * tt : 64 * (tt + 1)]
                        .rearrange("p (o d) -> p o d", o=1)
                        .broadcast_to([128, HLOC, 64])
                    )
                    nc.vector.tensor_mul(outv, psv, cb)
                    tmp = tmp_pool.tile([128, 256], f32, tag="tmp")
                    tmpv = tmp[:].rearrange("p (h d) -> p h d", d=32)
                    s0 = (
                        s_sb[:, 32 * tt : 32 * tt + 16]
                        .rearrange("p (o d) -> p o d", o=1)
                        .broadcast_to([128, HLOC, 16])
                    )
                    s1 = (
                        s_sb[:, 32 * tt + 16 : 32 * tt + 32]
                        .rearrange("p (o d) -> p o d", o=1)
                        .broadcast_to([128, HLOC, 16])
                    )
                    nc.vector.tensor_mul(tmpv[:, :, 0:16], psv[:, :, 16:32], s0)
                    nc.vector.tensor_mul(tmpv[:, :, 16:32], psv[:, :, 0:16], s1)
                    rotslice = outv[:, :, 0:32]
                    nc.vector.tensor_add(rotslice, rotslice, tmpv)
                    if X != "v":
                        dest = kT if X == "k" else qT
                        for d4 in range(PT):
                            tp = tr_psum.tile([128, 128], f32r, tag="tp")
                            nc.tensor.transpose(
                                tp[:], xn[:, 128 * d4 : 128 * (d4 + 1)], ident[:]
                            )
                            nc.vector.tensor_copy(
                                dest[:, S * d4 + 128 * tt : S * d4 + 128 * (tt + 1)],
                                tp[:],
                            )

    # ---------------- phase 2: attention + output projection -------------
    with tc.tile_pool(name="m1p", bufs=1) as m1_pool, tc.tile_pool(
        name="exp", bufs=6
    ) as exp_pool, tc.tile_pool(name="onorm", bufs=8) as onorm_pool, tc.tile_pool(
        name="rz", bufs=4
    ) as rz_pool, tc.tile_pool(name="zsb", bufs=2) as zsb_pool, tc.tile_pool(
        name="stg", bufs=2
    ) as stg_pool, tc.tile_pool(name="wos", bufs=2) as wos_pool, tc.tile_pool(
        name="ps_s", bufs=2, space="PSUM"
    ) as s_psum, tc.tile_pool(
        name="ps_o", bufs=2, space="PSUM"
    ) as out_psum, tc.tile_pool(
        name="ps_z", bufs=1, space="PSUM"
    ) as z_psum, tc.tile_pool(
        name="ps_w", bufs=1, space="PSUM"
    ) as wo_psum:
        m1_sb = m1_pool.tile([128, PT * E], f32r)
        nc.sync.dma_start(
            m1_sb[:].rearrange("p (t n) -> p t n", n=E),
            m1.ap().rearrange("(t p) n -> p t n", p=128),
        )
        for qc in range(QC):
            o_norm = []
            for p in range(PT):
                outA = out_psum.tile([65, 512], f32, tag="outA", bufs=1)
                outB = out_psum.tile([65, 512], f32, tag="outB", bufs=1)
                for kc in range(KC):
                    sps = s_psum.tile([128, 1024], f32, tag="sps")
                    nc.tensor.matmul(
                        sps[:, 0:512],
                        kT[0:64, S * p + 128 * kc : S * p + 128 * (kc + 1)],
                        qT[0:64, S * p + 512 * qc : S * p + 512 * (qc + 1)],
                        start=True,
                        stop=True,
                        tile_position=(0, 0),
                    )
                    nc.tensor.matmul(
                        sps[:, 512:1024],
                        kT[64:128, S * p + 128 * kc : S * p + 128 * (kc + 1)],
                        qT[64:128, S * p + 512 * qc : S * p + 512 * (qc + 1)],
                        start=True,
                        stop=True,
                        tile_position=(64, 0),
                    )
                    ex = exp_pool.tile([128, 1024], f32r, tag="ex")
                    nc.scalar.activation(ex[:], sps[:], FT.Exp)
                    nc.tensor.matmul(
                        outA[:],
                        vext[:, 520 * kc + 65 * (2 * p) : 520 * kc + 65 * (2 * p) + 65],
                        ex[:, 0:512],
                        start=(kc == 0),
                        stop=(kc == KC - 1),
                    )
                    nc.tensor.matmul(
                        outB[:],
                        vext[
                            :,
                            520 * kc + 65 * (2 * p + 1) : 520 * kc + 65 * (2 * p + 1) + 65,
                        ],
                        ex[:, 512:1024],
                        start=(kc == 0),
                        stop=(kc == KC - 1),
                    )
                o_n = onorm_pool.tile([128, 512], f32r, tag="on")
                for hh, outps in ((0, outA), (1, outB)):
                    rz = rz_pool.tile([65, 512], f32r, tag="rz")
                    with nc.allow_low_precision(reason="f32r softmax denom recip"):
                        nc.vector.reciprocal(rz[64:65, :], outps[64:65, :])
                    zps = z_psum.tile([64, 512], f32, tag="zps")
                    nc.tensor.matmul(
                        zps[:],
                        ones_t[64:65, 0:64],
                        rz[64:65, :],
                        start=True,
                        stop=True,
                        tile_position=(64, 0),
                    )
                    zsb = zsb_pool.tile([64, 512], f32, tag="zsb")
                    nc.vector.tensor_copy(zsb[:], zps[:])
                    if hh == 0:
                        nc.vector.tensor_mul(o_n[0:64, :], outps[0:64, :], zsb[:])
                    else:
                        st = stg_pool.tile([64, 512], f32r, tag="st")
                        nc.vector.tensor_mul(st[:], outps[0:64, :], zsb[:])
                        nc.sync.dma_start(o_n[64:128, :], st[:])
                o_norm.append(o_n)
            for m in range(E // 128):
                wps = wo_psum.tile([128, 512], f32, tag="wps")
                for p in range(PT):
                    nc.tensor.matmul(
                        wps[:],
                        m1_sb[:, E * p + 128 * m : E * p + 128 * (m + 1)],
                        o_norm[p][:],
                        start=(p == 0),
                        stop=(p == PT - 1),
                    )
                ws = wos_pool.tile([128, 512], f32, tag="ws")
                nc.vector.tensor_copy(ws[:], wps[:])
                nc.sync.dma_start(
                    part.ap()[128 * m : 128 * (m + 1), 512 * qc : 512 * (qc + 1)], ws[:]
                )


_NC_CACHE = {}


def _get_nc():
    if "nc" in _NC_CACHE:
        return _NC_CACHE["nc"]
    nc = bacc.Bacc("TRN2", target_bir_lowering=False, debug=False, num_devices=N_CORES)
    hidT = nc.dram_tensor("hidT", [E, S], f32r, kind="ExternalInput")
    wq = nc.dram_tensor("wq", [E, HS], f32r, kind="ExternalInput")
    wk = nc.dram_tensor("wk", [E, HS], f32r, kind="ExternalInput")
    wv = nc.dram_tensor("wv", [E, HS], f32r, kind="ExternalInput")
    m1 = nc.dram_tensor("m1", [HS, E], f32r, kind="ExternalInput")
    cmat = nc.dram_tensor("cmat", [S, 64], f32, kind="ExternalInput")
    smat = nc.dram_tensor("smat", [S, 32], f32, kind="ExternalInput")
    ident_in = nc.dram_tensor("ident", [128, 128], f32r, kind="ExternalInput")
    ones_in = nc.dram_tensor("ones", [128, 64], f32r, kind="ExternalInput")
    part = nc.dram_tensor("part", [E, S], f32, kind="ExternalOutput")
    from contextlib import ExitStack

    with tile.TileContext(nc) as tc, ExitStack() as ctx:
        _emit(nc, tc, ctx, (hidT, wq, wk, wv, m1, cmat, smat, ident_in, ones_in, part))
    nc.compile()
    _NC_CACHE["nc"] = nc
    return nc


def _in_maps(hidden_states, rotary_pos_emb, Wq, Wk, Wv, Wo):
    scale = np.float32(D**-0.5)
    f = np.asarray(rotary_pos_emb, np.float32)[0]  # [S, ROT]
    cmat = np.ones((S, 64), np.float32)
    cmat[:, 0:ROT] = np.cos(f)
    smat = np.empty((S, ROT), np.float32)
    smat[:, 0:16] = -np.sin(f[:, 0:16])
    smat[:, 16:ROT] = np.sin(f[:, 16:ROT])
    ident = np.eye(128, dtype=np.float32)
    ones = np.ones((128, 64), np.float32)
    hs = np.asarray(hidden_states, np.float32)
    Wq, Wk, Wv, Wo = (np.asarray(w, np.float32) for w in (Wq, Wk, Wv, Wo))
    maps = []
    for c in range(N_CORES):
        b, hg = divmod(c, 2)
        rows = slice(hg * HS, (hg + 1) * HS)
        maps.append(
            {
                "hidT": np.ascontiguousarray(hs[b].T),
                "wq": np.ascontiguousarray((Wq[rows] * scale).T),
                "wk": np.ascontiguousarray(Wk[rows].T),
                "wv": np.ascontiguousarray(Wv[rows].T),
                "m1": np.ascontiguousarray(Wo[:, rows].T),
                "cmat": cmat,
                "smat": smat,
                "ident": ident,
                "ones": ones,
            }
        )
    return maps


def kernel(hidden_states, rotary_pos_emb, Wq, Wk, Wv, Wo, bo, _trace=False):
    nc = _get_nc()
    maps = _in_maps(hidden_states, rotary_pos_emb, Wq, Wk, Wv, Wo)
    res = run_bass_kernel_spmd(
        nc, maps, core_ids=list(range(N_CORES)), trace=_trace
    )
    out = np.empty((B, S, E), np.float32)
    bo = np.asarray(bo, np.float32)
    for b in range(B):
        p0 = np.asarray(res.results[2 * b]["part"])
        p1 = np.asarray(res.results[2 * b + 1]["part"])
        out[b] = (p0 + p1).T + bo
    if _trace:
        kernel._last_results = res
    return out


# revision 6
# speedup vs baseline: 1.0719x; 1.0719x over previous
"""CLVP attention kernel for 8 Trainium2 NeuronCores.

Sharding: core c = 2*b + hg handles batch b (2048 tokens) and head-group hg
(8 of 16 heads).  Each core computes q/k/v projections for its heads, partial
rotary, attention, and a partial output projection over its heads' dims; the
host sums the two head-group partials per batch and adds the bias.

On-core layout highlights:
  - projections run in "natural" [token, dim] layout so partial rotary is a
    few strided DVE ops; q/k are then PE-transposed to [dim, token] for QK^T.
  - scores are computed transposed (S^T = [k_tok, q_tok]) so softmax's
    denominator and attn@v both contract over the partition axis on PE.
  - exp via ScalarE from PSUM, no max-subtraction (logits are O(1) here).
  - the v matmul uses a ones-extended stationary operand [v | 1] so row 64 of
    the accumulator is the softmax denominator for free.
  - everything runs in float32r (full-speed PE fp32 mode, ~1e-4 rounding).
"""

import numpy as np

import concourse.bass as bass
import concourse.tile as tile
from concourse import bacc, mybir
from concourse.bass_utils import run_bass_kernel_spmd

B, S, E, H, D, ROT = 4, 2048, 1024, 16, 64, 32
HLOC = 8            # heads per core
HS = HLOC * D       # 512 head dims per core
N_CORES = 8
KE = E // 128       # 8 contraction tiles for projections
TT = S // 128       # 16 token tiles
QC = S // 512       # 4 q chunks
KC = S // 128       # 16 k chunks
PT = HS // 128      # 4 pair-tiles (2 heads each)

f32 = mybir.dt.float32
f32r = mybir.dt.float32r
FT = mybir.ActivationFunctionType


def _emit(nc, tc, ctx, t):
    hidT, wq, wk, wv, m1, cmat, smat, ident_in, ones_in, part = t
    w_dram = {"q": wq, "k": wk, "v": wv}

    const = ctx.enter_context(tc.tile_pool(name="const", bufs=1))
    ident = const.tile([128, 128], f32r)
    nc.sync.dma_start(ident[:], ident_in.ap())
    ones_t = const.tile([128, 64], f32r)
    nc.sync.dma_start(ones_t[:], ones_in.ap())
    c_sb = const.tile([128, TT * 64], f32)
    nc.sync.dma_start(
        c_sb[:].rearrange("p (t d) -> p t d", d=64),
        cmat.ap().rearrange("(t p) d -> p t d", p=128),
    )
    s_sb = const.tile([128, TT * 32], f32)
    nc.sync.dma_start(
        s_sb[:].rearrange("p (t d) -> p t d", d=32),
        smat.ap().rearrange("(t p) d -> p t d", p=128),
    )
    # v_ext: [k-tile kc][head h][65] ; col 64 of each slot stays 1.0
    vext = const.tile([128, KC * HLOC * 65], f32r)
    nc.vector.tensor_copy(
        vext[:].rearrange("p (s c) -> p s c", c=65)[:, :, 64:65],
        ones_t[:, 0:1].rearrange("p (o c) -> p o c", o=1).broadcast_to(
            [128, KC * HLOC, 1]
        ),
    )
    kT = const.tile([128, PT * S], f32r)   # [pair-tile][token]
    qT = const.tile([128, PT * S], f32r)

    # ---------------- phase 1: projections + rotary + transpose ----------
    with tc.tile_pool(name="wts", bufs=1) as w_pool, tc.tile_pool(
        name="hid", bufs=16
    ) as hid_pool, tc.tile_pool(name="xnat", bufs=4) as xnat_pool, tc.tile_pool(
        name="tmp", bufs=4
    ) as tmp_pool, tc.tile_pool(
        name="pproj", bufs=6, space="PSUM"
    ) as proj_psum, tc.tile_pool(
        name="ptr", bufs=2, space="PSUM"
    ) as tr_psum:
        w_sb = {}
        for name in ("q", "k", "v"):
            wt = w_pool.tile([128, KE * HS], f32r, name=f"w{name}", tag=f"w{name}")
            nc.sync.dma_start(
                wt[:].rearrange("p (k n) -> p k n", n=HS),
                w_dram[name].ap().rearrange("(k p) n -> p k n", p=128),
            )
            w_sb[name] = wt

        for c8 in range(S // 256):
            hid_sl = []
            for k in range(KE):
                ht = hid_pool.tile([128, 256], f32r, tag="hid")
                nc.sync.dma_start(
                    ht[:], hidT.ap()[128 * k : 128 * (k + 1), 256 * c8 : 256 * c8 + 256]
                )
                hid_sl.append(ht)
            for X in ("k", "v", "q"):
                for t2 in range(2):
                    tt = 2 * c8 + t2
                    ps = proj_psum.tile([128, HS], f32, tag="pp")
                    for k in range(KE):
                        nc.tensor.matmul(
                            ps[:],
                            hid_sl[k][:, 128 * t2 : 128 * t2 + 128],
                            w_sb[X][:, HS * k : HS * (k + 1)],
                            start=(k == 0),
                            stop=(k == KE - 1),
                        )
                    psv = ps[:].rearrange("p (h d) -> p h d", d=64)
                    if X == "v":
                        blk = vext[:, 520 * tt : 520 * (tt + 1)]
                        outv = blk.rearrange("p (h c) -> p h c", c=65)[:, :, 0:64]
                    else:
                        xn = xnat_pool.tile([128, HS], f32r, tag="xn")
                        outv = xn[:].rearrange("p (h d) -> p h d", d=64)
                    cb = (
                        c_sb[:, 64 * tt : 64 * (tt + 1)]
                        .rearrange("p (o d) -> p o d", o=1)
                        .broadcast_to([128, HLOC, 64])
                    )
                    nc.vector.tensor_mul(outv, psv, cb)
                    tmp = tmp_pool.tile([128, 256], f32, tag="tmp")
                    tmpv = tmp[:].rearrange("p (h d) -> p h d", d=32)
                    s0 = (
                        s_sb[:, 32 * tt : 32 * tt + 16]
                        .rearrange("p (o d) -> p o d", o=1)
                        .broadcast_to([128, HLOC, 16])
                    )
                    s1 = (
                        s_sb[:, 32 * tt + 16 : 32 * tt + 32]
                        .rearrange("p (o d) -> p o d", o=1)
                        .broadcast_to([128, HLOC, 16])
                    )
                    nc.vector.tensor_mul(tmpv[:, :, 0:16], psv[:, :, 16:32], s0)
                    nc.vector.tensor_mul(tmpv[:, :, 16:32], psv[:, :, 0:16], s1)
                    rotslice = outv[:, :, 0:32]
                    nc.vector.tensor_add(rotslice, rotslice, tmpv)
                    if X != "v":
                        dest = kT if X == "k" else qT
                        for d4 in range(PT):
                            tp = tr_psum.tile([128, 128], f32r, tag="tp")
                            nc.tensor.transpose(
                                tp[:], xn[:, 128 * d4 : 128 * (d4 + 1)], ident[:]
                            )
                            # ScalarE is idle in phase 1; keep DVE for rotary
                            nc.scalar.copy(
                                dest[:, S * d4 + 128 * tt : S * d4 + 128 * (tt + 1)],
                                tp[:],
                            )

    # ---------------- phase 2: attention + output projection -------------
    with tc.tile_pool(name="m1p", bufs=1) as m1_pool, tc.tile_pool(
        name="exp", bufs=6
    ) as exp_pool, tc.tile_pool(name="onorm", bufs=8) as onorm_pool, tc.tile_pool(
        name="rz", bufs=4
    ) as rz_pool, tc.tile_pool(name="zsb", bufs=2) as zsb_pool, tc.tile_pool(
        name="stg", bufs=2
    ) as stg_pool, tc.tile_pool(name="wos", bufs=2) as wos_pool, tc.tile_pool(
        name="ps_s", bufs=2, space="PSUM"
    ) as s_psum, tc.tile_pool(
        name="ps_o", bufs=2, space="PSUM"
    ) as out_psum, tc.tile_pool(
        name="ps_z", bufs=1, space="PSUM"
    ) as z_psum, tc.tile_pool(
        name="ps_w", bufs=1, space="PSUM"
    ) as wo_psum:
        m1_sb = m1_pool.tile([128, PT * E], f32r)
        nc.sync.dma_start(
            m1_sb[:].rearrange("p (t n) -> p t n", n=E),
            m1.ap().rearrange("(t p) n -> p t n", p=128),
        )
        for qc in range(QC):
            o_norm = []
            for p in range(PT):
                outA = out_psum.tile([65, 512], f32, tag="outA", bufs=1)
                outB = out_psum.tile([65, 512], f32, tag="outB", bufs=1)
                for kc in range(KC):
                    sps = s_psum.tile([128, 1024], f32, tag="sps")
                    nc.tensor.matmul(
                        sps[:, 0:512],
                        kT[0:64, S * p + 128 * kc : S * p + 128 * (kc + 1)],
                        qT[0:64, S * p + 512 * qc : S * p + 512 * (qc + 1)],
                        start=True,
                        stop=True,
                        tile_position=(0, 0),
                    )
                    nc.tensor.matmul(
                        sps[:, 512:1024],
                        kT[64:128, S * p + 128 * kc : S * p + 128 * (kc + 1)],
                        qT[64:128, S * p + 512 * qc : S * p + 512 * (qc + 1)],
                        start=True,
                        stop=True,
                        tile_position=(64, 0),
                    )
                    ex = exp_pool.tile([128, 1024], f32r, tag="ex")
                    nc.scalar.activation(ex[:], sps[:], FT.Exp)
                    nc.tensor.matmul(
                        outA[:],
                        vext[:, 520 * kc + 65 * (2 * p) : 520 * kc + 65 * (2 * p) + 65],
                        ex[:, 0:512],
                        start=(kc == 0),
                        stop=(kc == KC - 1),
                    )
                    nc.tensor.matmul(
                        outB[:],
                        vext[
                            :,
                            520 * kc + 65 * (2 * p + 1) : 520 * kc + 65 * (2 * p + 1) + 65,
                        ],
                        ex[:, 512:1024],
                        start=(kc == 0),
                        stop=(kc == KC - 1),
                    )
                o_n = onorm_pool.tile([128, 512], f32r, tag="on")
                for hh, outps in ((0, outA), (1, outB)):
                    rz = rz_pool.tile([65, 512], f32r, tag="rz")
                    with nc.allow_low_precision(reason="f32r softmax denom recip"):
                        nc.vector.reciprocal(rz[64:65, :], outps[64:65, :])
                    zps = z_psum.tile([64, 512], f32, tag="zps")
                    nc.tensor.matmul(
                        zps[:],
                        ones_t[64:65, 0:64],
                        rz[64:65, :],
                        start=True,
                        stop=True,
                        tile_position=(64, 0),
                    )
                    zsb = zsb_pool.tile([64, 512], f32, tag="zsb")
                    nc.vector.tensor_copy(zsb[:], zps[:])
                    if hh == 0:
                        nc.vector.tensor_mul(o_n[0:64, :], outps[0:64, :], zsb[:])
                    else:
                        st = stg_pool.tile([64, 512], f32r, tag="st")
                        nc.vector.tensor_mul(st[:], outps[0:64, :], zsb[:])
                        nc.sync.dma_start(o_n[64:128, :], st[:])
                o_norm.append(o_n)
            for m in range(E // 128):
                wps = wo_psum.tile([128, 512], f32, tag="wps")
                for p in range(PT):
                    nc.tensor.matmul(
                        wps[:],
                        m1_sb[:, E * p + 128 * m : E * p + 128 * (m + 1)],
                        o_norm[p][:],
                        start=(p == 0),
                        stop=(p == PT - 1),
                    )
                ws = wos_pool.tile([128, 512], f32, tag="ws")
                nc.vector.tensor_copy(ws[:], wps[:])
                nc.sync.dma_start(
                    part.ap()[128 * m : 128 * (m + 1), 512 * qc : 512 * (qc + 1)], ws[:]
                )


_NC_CACHE = {}


def _get_nc():
    if "nc" in _NC_CACHE:
        return _NC_CACHE["nc"]
    nc = bacc.Bacc("TRN2", target_bir_lowering=False, debug=False, num_devices=N_CORES)
    hidT = nc.dram_tensor("hidT", [E, S], f32r, kind="ExternalInput")
    wq = nc.dram_tensor("wq", [E, HS], f32r, kind="ExternalInput")
    wk = nc.dram_tensor("wk", [E, HS], f32r, kind="ExternalInput")
    wv = nc.dram_tensor("wv", [E, HS], f32r, kind="ExternalInput")
    m1 = nc.dram_tensor("m1", [HS, E], f32r, kind="ExternalInput")
    cmat = nc.dram_tensor("cmat", [S, 64], f32, kind="ExternalInput")
    smat = nc.dram_tensor("smat", [S, 32], f32, kind="ExternalInput")
    ident_in = nc.dram_tensor("ident", [128, 128], f32r, kind="ExternalInput")
    ones_in = nc.dram_tensor("ones", [128, 64], f32r, kind="ExternalInput")
    part = nc.dram_tensor("part", [E, S], f32, kind="ExternalOutput")
    from contextlib import ExitStack

    with tile.TileContext(nc) as tc, ExitStack() as ctx:
        _emit(nc, tc, ctx, (hidT, wq, wk, wv, m1, cmat, smat, ident_in, ones_in, part))
    nc.compile()
    _NC_CACHE["nc"] = nc
    return nc


def _in_maps(hidden_states, rotary_pos_emb, Wq, Wk, Wv, Wo):
    scale = np.float32(D**-0.5)
    f = np.asarray(rotary_pos_emb, np.float32)[0]  # [S, ROT]
    cmat = np.ones((S, 64), np.float32)
    cmat[:, 0:ROT] = np.cos(f)
    smat = np.empty((S, ROT), np.float32)
    smat[:, 0:16] = -np.sin(f[:, 0:16])
    smat[:, 16:ROT] = np.sin(f[:, 16:ROT])
    ident = np.eye(128, dtype=np.float32)
    ones = np.ones((128, 64), np.float32)
    hs = np.asarray(hidden_states, np.float32)
    Wq, Wk, Wv, Wo = (np.asarray(w, np.float32) for w in (Wq, Wk, Wv, Wo))
    maps = []
    for c in range(N_CORES):
        b, hg = divmod(c, 2)
        rows = slice(hg * HS, (hg + 1) * HS)
        maps.append(
            {
                "hidT": np.ascontiguousarray(hs[b].T),
                "wq": np.ascontiguousarray((Wq[rows] * scale).T),
                "wk": np.ascontiguousarray(Wk[rows].T),
                "wv": np.ascontiguousarray(Wv[rows].T),
                "m1": np.ascontiguousarray(Wo[:, rows].T),
                "cmat": cmat,
                "smat": smat,
                "ident": ident,
                "ones": ones,
            }
        )
    return maps


def kernel(hidden_states, rotary_pos_emb, Wq, Wk, Wv, Wo, bo, _trace=False):
    nc = _get_nc()
    maps = _in_maps(hidden_states, rotary_pos_emb, Wq, Wk, Wv, Wo)
    res = run_bass_kernel_spmd(
        nc, maps, core_ids=list(range(N_CORES)), trace=_trace
    )
    out = np.empty((B, S, E), np.float32)
    bo = np.asarray(bo, np.float32)
    for b in range(B):
        p0 = np.asarray(res.results[2 * b]["part"])
        p1 = np.asarray(res.results[2 * b + 1]["part"])
        out[b] = (p0 + p1).T + bo
    if _trace:
        kernel._last_results = res
    return out
